# revision 18
# baseline (speedup 1.0000x reference)
"""Cross-attention (GQA) Trainium2 Bass kernel.

Problem: B=2, Tq=Tkv=2048, D_MODEL=1024, 16 query heads / 4 kv heads,
head_dim=64.  Sharded over 8 NeuronCores as batch(2) x kv-group(4); each
core computes 4 query heads + its single kv head and a partial output
projection (Wo row-split by head group); partials are summed on host.

On-chip dataflow keeps activations "transposed" (feature dim on SBUF
partitions) end-to-end so that scores, softmax and P@V need no on-chip
transposes of large tensors:

  A: qT[e,t] = WqT.T @ xqT,  kvT = WkvT.T @ xcT        (fp32r, N=512)
     v[tk,dv] via PE-transpose of vT tiles
  B: ST[tk,tq] = kT.T @ qT_h ; two heads packed in the PE array via
     row-groups (K=64 each, h_even rows 0-63, h_odd rows 64-127)
  C: P = exp(ST/8)  on ScalarE, PSUM->SBUF, 1024-wide instructions
  D: outT'[dv+sum,tq] = [v|1].T @ P ; the ones-column matmul is
     col-packed into a spare PE column-group => denominators come out
     of the same pass.  h_odd heads are placed at partitions 64..127.
  E: yT += WoT_pair.T @ outT_norm (K=128: two heads stacked)
"""

import os
import sys

import numpy as np

for _p in ("/opt/trn_rl_repo",):
    if _p not in sys.path and os.path.isdir(_p):
        sys.path.insert(0, _p)

import concourse.bass as bass
import concourse.bacc as bacc
import concourse.mybir as mybir
from concourse.tile import TileContext

# ---------------------------------------------------------------- problem dims
B = 2
TQ = 2048
TKV = 2048
D_MODEL = 1024
N_HEADS = 16
N_KV_HEADS = 4
HEAD_DIM = 64
N_CORES = 8
GROUPS = N_KV_HEADS  # kv groups = 4
HEADS_PER_DEV = N_HEADS // GROUPS  # 4
DQ = HEADS_PER_DEV * HEAD_DIM  # 256
DKV = 2 * HEAD_DIM  # 128 (k rows + v rows stacked)
SCALE = 1.0 / float(np.sqrt(HEAD_DIM))

P = 128
FREE = 512  # matmul moving-operand chunk
BLK = 1024  # tq block width (exp instruction width)

F32 = mybir.dt.float32
F32R = mybir.dt.float32r
F16 = mybir.dt.float16


def build_bass():
    nc = bacc.Bacc()

    xq = nc.declare_dram_parameter("xqT", [D_MODEL, TQ], F16, isOutput=False)
    xc = nc.declare_dram_parameter("xcT", [D_MODEL, TKV], F16, isOutput=False)
    wq = nc.declare_dram_parameter("wqT", [D_MODEL, DQ], F16, isOutput=False)
    wkv = nc.declare_dram_parameter("wkvT", [D_MODEL, DKV], F16, isOutput=False)
    wo = nc.declare_dram_parameter("woT", [DQ, D_MODEL], F16, isOutput=False)
    cid = nc.declare_dram_parameter("cid", [P, P + 64], F16, isOutput=False)
    yt = nc.declare_dram_parameter("yT", [D_MODEL, TQ], F32, isOutput=True)

    DT = D_MODEL // P  # 8 d-tiles
    ET = DQ // P  # 2 e-tiles (query head pairs)
    NCH = TQ // FREE  # 4 chunks of 512
    NTK = TKV // P  # 16 tk tiles
    NBLK = TQ // BLK  # 2 tq blocks
    JPB = BLK // FREE  # 2 free-chunks per block
    MT = D_MODEL // P  # 8 output m-tiles

    with TileContext(nc) as tc:
        with (
            tc.tile_pool(name="consts", bufs=1) as consts,
            tc.tile_pool(name="xch", bufs=3) as xpool,
            tc.tile_pool(name="pt", bufs=6) as ptpool,
            tc.tile_pool(name="nrm", bufs=2) as nrmpool,
            tc.tile_pool(name="yout", bufs=3) as ypool,
            tc.tile_pool(name="psA", bufs=2, space="PSUM") as psA,
            tc.tile_pool(name="psB", bufs=2, space="PSUM") as psB,
        ):
            # ---------------- constants / persistent tiles
            ident = consts.tile([P, P + 64], F16, tag="ident")
            nc.sync.dma_start(ident, cid[:])
            ones = ident[:, P : P + 64]

            wq_sb = consts.tile([P, DT, DQ], F16, tag="wq")
            nc.sync.dma_start(wq_sb, wq.rearrange("(i p) e -> p i e", p=P))
            wkv_sb = consts.tile([P, DT, DKV], F16, tag="wkv")
            nc.sync.dma_start(wkv_sb, wkv.rearrange("(i p) e -> p i e", p=P))
            wo_sb = consts.tile([P, ET, D_MODEL], F16, tag="wo")
            nc.sync.dma_start(wo_sb, wo.rearrange("(i p) m -> p i m", p=P))

            qt = consts.tile([P, ET, TQ], F16, tag="qt")  # qT: heads 2/tile
            kv = consts.tile([P, TKV], F16, tag="kv")  # rows 0-63 kT, 64-127 vT
            k2 = consts.tile([P, TKV], F16, tag="k2")  # rows 64-127 = kT copy
            vp = consts.tile([P, NTK, P], F16, tag="vp")  # [v | ones]
            vp2 = consts.tile([P, NTK, P], F16, tag="vp2")  # [ones | v]
            outs = consts.tile([P, ET, TQ], F16, tag="outs")  # normalized outT

            # ---------------- stage A: projections (weights stationary)
            # kv first (every BCD iteration needs the full kT/vT), then q
            for c in range(NCH):
                cs = slice(c * FREE, (c + 1) * FREE)
                xc_t = xpool.tile([P, DT, FREE], F16, tag="xch")
                nc.sync.dma_start(
                    xc_t, xc.rearrange("(i p) t -> p i t", p=P)[:, :, cs]
                )
                pkv = psB.tile([P, FREE], F32, tag="psB")
                for i in range(DT):
                    nc.tensor.matmul(
                        pkv,
                        (wkv_sb[:, i, :]),
                        (xc_t[:, i, :]),
                        start=(i == 0),
                        stop=(i == DT - 1),
                    )
                nc.vector.tensor_copy(kv[:, cs], pkv)
                # duplicate kT rows into partitions 64..127 for row-packing
                nc.sync.dma_start(k2[HEAD_DIM : 2 * HEAD_DIM, cs], kv[:HEAD_DIM, cs])

            def emit_q_chunk(c):
                cs = slice(c * FREE, (c + 1) * FREE)
                xq_t = xpool.tile([P, DT, FREE], F16, tag="xch", name="xq_t")
                nc.sync.dma_start(
                    xq_t, xq.rearrange("(i p) t -> p i t", p=P)[:, :, cs]
                )
                for e in range(ET):
                    pq = psA.tile([P, FREE], F32, tag="psA", name="pq")
                    for i in range(DT):
                        nc.tensor.matmul(
                            pq,
                            (wq_sb[:, i, e * P : (e + 1) * P]),
                            (xq_t[:, i, :]),
                            start=(i == 0),
                            stop=(i == DT - 1),
                        )
                    nc.vector.tensor_copy(qt[:, e, cs], pq)

            for _c in range(min(2, NCH)):
                emit_q_chunk(_c)

            # v' tiles: PE-transpose vT[64, tk*128 ..] -> [128, 64], then
            # build [v | ones] (for even heads) and [ones | v] (odd heads).
            # The all-ones half makes the same matmul emit the softmax
            # denominators, replicated across 64 partitions.
            for t in range(NTK):
                ts_ = slice(t * P, (t + 1) * P)
                pv = psB.tile([P, HEAD_DIM], F16, tag="psB")
                nc.tensor.transpose(
                    pv, kv[HEAD_DIM : 2 * HEAD_DIM, ts_], ident[HEAD_DIM:, HEAD_DIM:P]
                )
                nc.vector.tensor_copy(vp[:, t, :HEAD_DIM], pv)
                nc.vector.tensor_copy(vp2[:, t, HEAD_DIM:], pv)
                nc.vector.tensor_copy(vp[:, t, HEAD_DIM:], ones)
                nc.vector.tensor_copy(vp2[:, t, :HEAD_DIM], ones)

            # ---------------- stages B/C/D: attention per head-pair
            first_bcd = True
            for e in range(ET):  # head pair (h_even=2e, h_odd=2e+1)
                for blk in range(NBLK):
                    bs = slice(blk * BLK, (blk + 1) * BLK)
                    pd = [
                        psB.tile([P, BLK], F32, tag="psB", name=f"pd{_h}")
                        for _h in range(2)
                    ]  # D accumulators: [0]=h_even rows 0-64, [1]=h_odd
                    for t in range(NTK):
                        ts_ = slice(t * P, (t + 1) * P)
                        pb = [
                            psA.tile([P, BLK], F32, tag="psA", name=f"pb{_h}")
                            for _h in range(2)
                        ]
                        for j in range(JPB):
                            js = slice(blk * BLK + j * FREE, blk * BLK + (j + 1) * FREE)
                            jo = slice(j * FREE, (j + 1) * FREE)
                            # scores, 2 heads row-packed (K=64 each)
                            nc.tensor.matmul(
                                pb[0][:, jo],
                                (kv[:HEAD_DIM, ts_]),
                                (qt[:HEAD_DIM, e, js]),
                            )
                            nc.tensor.matmul(
                                pb[1][:, jo],
                                (k2[HEAD_DIM:, ts_]),
                                (qt[HEAD_DIM:, e, js]),
                            )
                        for h in range(2):
                            pt = ptpool.tile([P, BLK], F16, tag="pt")
                            nc.scalar.activation(
                                pt,
                                pb[h],
                                mybir.ActivationFunctionType.Exp,
                                bias=0.0,
                                scale=SCALE,
                            )
                            # M=128 stationary [v|ones] / [ones|v]: one
                            # matmul per head yields out_h in its 64-row
                            # half and the softmax denominators (replicated
                            # x64) in the other half.  dst base stays 0
                            # (fp32r matmuls cannot target offset psum
                            # partitions).
                            vo = vp if h == 0 else vp2
                            for j in range(JPB):
                                jo = slice(j * FREE, (j + 1) * FREE)
                                nc.tensor.matmul(
                                    pd[h][:, jo],
                                    vo[:, t, :],
                                    pt[:, jo],
                                    start=(t == 0),
                                    stop=(t == NTK - 1),
                                    skip_group_check=True,
                                )
                    if first_bcd:
                        first_bcd = False
                        for _c in range(2, NCH):
                            emit_q_chunk(_c)
                    # spill raw accumulators to SBUF immediately (~1.2us)
                    # so the PSUM slots free up and the PE never stalls;
                    # the normalize chain below runs off the critical path.
                    for h in range(2):
                        raw = nrmpool.tile([P, BLK], F32, tag=f"raw{h}")
                        nc.vector.tensor_copy(raw, pd[h])
                        lo = slice(0, 64) if h == 0 else slice(64, 128)
                        hi = slice(64, 128) if h == 0 else slice(0, 64)
                        rec = nrmpool.tile([P, BLK], F32, tag="rec")
                        rec2 = nrmpool.tile([P, BLK], F32, tag="rec2")
                        nc.vector.reciprocal(rec[hi, :], raw[hi, :])
                        nc.sync.dma_start(rec2[lo, :], rec[hi, :])
                        nc.vector.tensor_mul(
                            outs[lo, e, bs], raw[lo, :], rec2[lo, :]
                        )

            # ---------------- stage E: output projection (partial)
            for m in range(MT):
                ms = slice(m * P, (m + 1) * P)
                for c in range(NCH):
                    cs = slice(c * FREE, (c + 1) * FREE)
                    py = psA.tile([P, FREE], F32, tag="psA")
                    for e in range(ET):
                        nc.tensor.matmul(
                            py,
                            (wo_sb[:, e, ms]),
                            (outs[:, e, cs]),
                            start=(e == 0),
                            stop=(e == ET - 1),
                        )
                    yo = ypool.tile([P, FREE], F32, tag="yout")
                    nc.vector.tensor_copy(yo, py)
                    nc.sync.dma_start(yt[ms, cs], yo)

    nc.finalize()  # Bacc: runs wait-splitting/reg-alloc passes
    return nc


_NC_CACHE = None


def _get_nc():
    global _NC_CACHE
    if _NC_CACHE is None:
        _NC_CACHE = build_bass()
    return _NC_CACHE


def _cid():
    c = np.zeros((P, P + 64), dtype=np.float16)
    c[:, :P] = np.eye(P, dtype=np.float32)
    c[:, P:] = 1.0
    return c


def shard_inputs(query, context, Wq, Wk, Wv, Wo):
    """host-side sharding: 8 cores = batch(2) x kv-group(4)"""
    in_maps = []
    xqT = [np.ascontiguousarray(query[b].T).astype(np.float16) for b in range(B)]
    xcT = [np.ascontiguousarray(context[b].T).astype(np.float16) for b in range(B)]
    for core in range(N_CORES):
        b, g = divmod(core, GROUPS)
        wqT = np.ascontiguousarray(Wq[g * DQ : (g + 1) * DQ, :].T).astype(np.float16)
        wkvT = np.ascontiguousarray(
            np.concatenate(
                [
                    Wk[g * HEAD_DIM : (g + 1) * HEAD_DIM, :],
                    Wv[g * HEAD_DIM : (g + 1) * HEAD_DIM, :],
                ],
                axis=0,
            ).T
        ).astype(np.float16)
        woT = np.ascontiguousarray(Wo[:, g * DQ : (g + 1) * DQ].T).astype(np.float16)
        in_maps.append(
            {
                "xqT": xqT[b],
                "xcT": xcT[b],
                "wqT": wqT,
                "wkvT": wkvT,
                "woT": woT,
                "cid": _cid(),
            }
        )
    return in_maps


def kernel(query, context, Wq, Wk, Wv, Wo, _want_profile=False):
    from concourse.bass_utils import run_bass_kernel_spmd

    nc = _get_nc()
    in_maps = shard_inputs(query, context, Wq, Wk, Wv, Wo)
    res = run_bass_kernel_spmd(
        nc, in_maps, core_ids=list(range(N_CORES)), trace=_want_profile
    )
    out = np.zeros((B, TQ, D_MODEL), dtype=np.float32)
    for core in range(N_CORES):
        b = core // GROUPS
        out[b] += res.results[core]["yT"].T
    if _want_profile:
        return out, res
    return out
